# revision 3
# baseline (speedup 1.0000x reference)
"""Bass/Trainium2 kernel for 3-layer bipartite GNN message passing (BPR).

Strategy (8 NeuronCores, SPMD single program):
  - Shard both node tables by destination range: core c owns herbs
    [c*H/8,(c+1)*H/8) and genes [c*G/8,(c+1)*G/8).
  - Each SpMM is computed destination-major: edges sorted by destination,
    grouped into "windows" of <=64 consecutive destination rows. Each window
    accumulates sum_e w[e] * x[src[e]] in PSUM via a one-hot matmul:
        S[e, q] = w[e] * (dstloc[e] == q)   (built on DVE, one op per chunk)
        psum[64, F] += S.T @ gathered_rows[128, F]
  - The degree residual (x*d) is folded in as one extra "self edge" per
    destination row, gathering from the same-side table. To allow one gather
    table per SpMM, each layer's state is a single combined table [herbs; genes].
  - Source rows are fetched with per-chunk indirect DMA gathers (128 rows of
    256B per call).
  - Window outputs are written to a per-core "staging" tensor (windows
    concatenated); the permutation is undone host-side, and the next layer's
    gather indices are precomputed in staging coordinates. Layers exchange
    state with a single AllGather of the staging tensor per layer.
"""

import os
import sys
import time
import numpy as np

sys.path.insert(0, "/opt/trn_rl_repo")

LAST_EXEC_NS = None

import concourse.bass as bass
import concourse.bacc as bacc
import concourse.mybir as mybir
import concourse.tile as tile

NCORES = 8
F = 64
W = 64            # destination window (one-hot width)
C_H = 19          # chunks per group, herb-destination SpMM (deg ~41 incl residual)
C_G = 4           # chunks per group, gene-destination SpMM (deg ~9 incl residual)
G_SUP = 16        # chunks per super-chunk (meta-load batching)
P = 128
N_QUEUES = 1      # SWDGE dynamic queues for indirect gathers (1..4)


def _build_side_layout(dst_local, order_src, order_w, n_dst_shard, C):
    """Greedy window/group layout for one SpMM side on one core.

    dst_local: sorted destination (shard-local) per edge [E]
    order_src/order_w: matching src ids (global, layer-independent meaning
      resolved later) and weights.
    Returns dict with slot arrays [T,128] (src slot as positions into the
    *edge list*, -1 for pad), dstloc/wv arrays, group base list, and
    loc[n_dst_shard] = staging row of each dst row.
    """
    E = len(dst_local)
    cap = C * P
    bases = []
    slot_edge = []  # per group: array of cap edge-ids (-1 pad)
    loc = np.full(n_dst_shard, -1, np.int64)
    i = 0
    while i < E:
        base = dst_local[i]
        g = len(bases)
        # edges while dst < base + W, at most cap, cutting at dst-run boundary
        j_max = min(i + cap, E)
        j = int(np.searchsorted(dst_local[i:j_max + 1], base + W)) + i
        j = min(j, j_max)
        if j < E and j == j_max and dst_local[j] == dst_local[j - 1]:
            # would split a dst run across groups: back off to run start
            run_start = int(np.searchsorted(dst_local[i:j], dst_local[j - 1])) + i
            assert run_start > i, "single dst run exceeds group capacity"
            j = run_start
        sl = np.full(cap, -1, np.int64)
        sl[:j - i] = np.arange(i, j)
        slot_edge.append(sl)
        span_rows = np.unique(dst_local[i:j])
        loc[span_rows] = g * W + (span_rows - base)
        bases.append(base)
        i = j
    n_groups = len(bases)
    return {
        "slot_edge": slot_edge, "bases": np.asarray(bases, np.int64),
        "loc": loc, "n_groups": n_groups,
        "dst_local": dst_local, "w": order_w, "src": order_src,
    }


def _finalize_side(layout, n_groups_common, C):
    """Pad to the common group count and emit [T,128] arrays."""
    cap = C * P
    n_g = n_groups_common
    T = n_g * C
    dstloc = np.zeros((T, P), np.float32)
    wv = np.zeros((T, P), np.float32)
    src_e = np.full((T, P), -1, np.int64)  # edge ids, -1 pad
    for g in range(layout["n_groups"]):
        sl = layout["slot_edge"][g]
        valid = sl >= 0
        dl = np.zeros(cap, np.float32)
        wvv = np.zeros(cap, np.float32)
        dl[valid] = (layout["dst_local"][sl[valid]] - layout["bases"][g]).astype(np.float32)
        wvv[valid] = layout["w"][sl[valid]]
        dstloc[g * C:(g + 1) * C] = dl.reshape(C, P)
        wv[g * C:(g + 1) * C] = wvv.reshape(C, P)
        src_e[g * C:(g + 1) * C] = sl.reshape(C, P)
    return dstloc, wv, src_e, T


def _to_super(a, n_sup, g_sup):
    """[T,128] -> [n_sup, 128, g_sup] (chunk t = s*g_sup + c at [s,:,c])."""
    T = a.shape[0]
    pad = n_sup * g_sup - T
    if pad:
        a = np.concatenate([a, np.zeros((pad, P), a.dtype)], axis=0)
    return np.ascontiguousarray(a.reshape(n_sup, g_sup, P).transpose(0, 2, 1))


def kernel(herbs_embedding, genes_embedding, d_i, d_j,
           edge_herb, edge_gene, w_hg, w_gh):
    herbs_embedding = np.asarray(herbs_embedding, np.float32)
    genes_embedding = np.asarray(genes_embedding, np.float32)
    d_i = np.asarray(d_i, np.float32)
    d_j = np.asarray(d_j, np.float32)
    edge_herb = np.asarray(edge_herb, np.int32)
    edge_gene = np.asarray(edge_gene, np.int32)
    w_hg = np.asarray(w_hg, np.float32)
    w_gh = np.asarray(w_gh, np.float32)

    NH, NG = herbs_embedding.shape[0], genes_embedding.shape[0]
    HPC, GPC = NH // NCORES, NG // NCORES
    COMB0 = NH + NG

    # ---------------- host-side layout, per core ----------------
    cores = []
    for c in range(NCORES):
        h_lo, h_hi = c * HPC, (c + 1) * HPC
        g_lo, g_hi = c * GPC, (c + 1) * GPC

        # h-SpMM: dst = herb in [h_lo,h_hi); src = gene (real) or herb (residual)
        m = (edge_herb >= h_lo) & (edge_herb < h_hi)
        e_dst = np.concatenate([edge_herb[m] - h_lo, np.arange(HPC)])
        # src id encodings: real gene j -> (1, j); residual herb i -> (0, i)
        e_side = np.concatenate([np.ones(m.sum(), np.int8), np.zeros(HPC, np.int8)])
        e_src = np.concatenate([edge_gene[m], np.arange(h_lo, h_hi)])
        e_w = np.concatenate([w_hg[m], d_i[h_lo:h_hi]])
        o = np.argsort(e_dst, kind="stable")
        lay_h = _build_side_layout(e_dst[o], (e_side[o], e_src[o]), e_w[o], HPC, C_H)

        # g-SpMM: dst = gene in [g_lo,g_hi); src = herb (real) or gene (residual)
        m2 = (edge_gene >= g_lo) & (edge_gene < g_hi)
        e_dst2 = np.concatenate([edge_gene[m2] - g_lo, np.arange(GPC)])
        e_side2 = np.concatenate([np.zeros(m2.sum(), np.int8), np.ones(GPC, np.int8)])
        e_src2 = np.concatenate([edge_herb[m2], np.arange(g_lo, g_hi)])
        e_w2 = np.concatenate([w_gh[m2], d_j[g_lo:g_hi]])
        o2 = np.argsort(e_dst2, kind="stable")
        lay_g = _build_side_layout(e_dst2[o2], (e_side2[o2], e_src2[o2]), e_w2[o2],
                                   GPC, C_G)
        cores.append({"h": lay_h, "g": lay_g})

    NGH = max(cc["h"]["n_groups"] for cc in cores)
    NGG = max(cc["g"]["n_groups"] for cc in cores)
    S_H, S_G = NGH * W, NGG * W
    S_TOT = S_H + S_G
    T_H, T_G = NGH * C_H, NGG * C_G
    NSUP_H = -(-T_H // G_SUP)
    NSUP_G = -(-T_G // G_SUP)

    # global staging position of each node, shared by layers 1/2 gather indexing
    hloc = np.empty(NH, np.int64)
    gloc = np.empty(NG, np.int64)
    for c in range(NCORES):
        hl = cores[c]["h"]["loc"]
        gl = cores[c]["g"]["loc"]
        assert (hl >= 0).all() and (gl >= 0).all()
        hloc[c * HPC:(c + 1) * HPC] = c * S_TOT + hl
        gloc[c * GPC:(c + 1) * GPC] = c * S_TOT + S_H + gl

    in_maps_np = []
    for c in range(NCORES):
        d = {}
        for side, C, T, nsup in (("h", C_H, T_H, NSUP_H), ("g", C_G, T_G, NSUP_G)):
            lay = cores[c][side]
            dstloc, wv, src_e, _ = _finalize_side(lay, T // C, C)
            e_side, e_src = lay["src"]
            valid = src_e >= 0
            sflat = np.zeros(src_e.shape, np.int64)
            side_flat = np.zeros(src_e.shape, np.int8)
            sflat[valid] = e_src[src_e[valid]]
            side_flat[valid] = e_side[src_e[valid]]
            # layer-1 combined table [h0; g0]: herb i -> i, gene j -> NH + j
            idx1 = np.where(side_flat == 0, sflat, NH + sflat).astype(np.int32)
            # layers 2/3: staging coords
            idx23 = np.where(side_flat == 0, hloc[np.clip(sflat, 0, NH - 1)],
                             gloc[np.clip(sflat, 0, NG - 1)]).astype(np.int32)
            idx1[~valid] = 0
            idx23[~valid] = 0
            d[f"idx1_{side}"] = _to_super(idx1, nsup, G_SUP)
            d[f"idx23_{side}"] = _to_super(idx23, nsup, G_SUP)
            d[f"dstloc_{side}"] = _to_super(dstloc, nsup, G_SUP)
            d[f"wv_{side}"] = _to_super(wv, nsup, G_SUP)
        d["comb0"] = np.concatenate([herbs_embedding, genes_embedding], axis=0)
        d["iota"] = np.tile(np.arange(W, dtype=np.float32), (P, 1))
        in_maps_np.append(d)

    # ---------------- device program ----------------
    nc = bacc.Bacc("TRN2", target_bir_lowering=False, debug=False,
                   num_devices=NCORES,
                   num_swdge_queues=N_QUEUES)
    f32, i32 = mybir.dt.float32, mybir.dt.int32

    comb0_d = nc.dram_tensor("comb0", [COMB0, F], f32, kind="ExternalInput")
    iota_d = nc.dram_tensor("iota", [P, W], f32, kind="ExternalInput")
    meta = {}
    for side, T, nsup in (("h", T_H, NSUP_H), ("g", T_G, NSUP_G)):
        for nm in ("idx1", "idx23", "dstloc", "wv"):
            dt = i32 if nm.startswith("idx") else f32
            meta[f"{nm}_{side}"] = nc.dram_tensor(
                f"{nm}_{side}", [nsup, P, G_SUP], dt, kind="ExternalInput")

    stag = [nc.dram_tensor(f"stag{l}", [S_TOT, F], f32,
                           kind="ExternalOutput") for l in (1, 2, 3)]
    # AG inputs cannot be kernel I/O: internal mirrors for layers 1,2
    stag_int = [nc.dram_tensor(f"stagint{l}", [S_TOT, F], f32, kind="Internal")
                for l in (1, 2)]
    comb_ag = [nc.dram_tensor(f"comb{l}", [NCORES * S_TOT, F], f32,
                              kind="Internal", addr_space="Shared")
               for l in (1, 2)]

    def emit_spmm(tc, pools, layer, side, table_ap, stag_ap, stag_off):
        C = C_H if side == "h" else C_G
        T = T_H if side == "h" else T_G
        nsup = NSUP_H if side == "h" else NSUP_G
        idx_name = ("idx1_" if layer == 1 else "idx23_") + side
        (meta_pool, rows_pool, sel_pool, out_pool, psum_pool, iota_t) = pools
        qrr = 0
        ps = None
        for s in range(nsup):
            idx_t = meta_pool.tile([P, G_SUP], i32, tag="mi")
            dst_t = meta_pool.tile([P, G_SUP], f32, tag="md")
            w_t = meta_pool.tile([P, G_SUP], f32, tag="mw")
            nc.sync.dma_start(out=idx_t[:], in_=meta[idx_name].ap()[s, :, :])
            nc.sync.dma_start(out=dst_t[:], in_=meta["dstloc_" + side].ap()[s, :, :])
            nc.sync.dma_start(out=w_t[:], in_=meta["wv_" + side].ap()[s, :, :])
            rows_t = rows_pool.tile([P, G_SUP * F], f32, tag="rows")
            for cch in range(G_SUP):
                t = s * G_SUP + cch
                if t >= T:
                    break
                nc.gpsimd.indirect_dma_start(
                    out=rows_t[:, cch * F:(cch + 1) * F],
                    out_offset=None,
                    in_=table_ap,
                    in_offset=bass.IndirectOffsetOnAxis(
                        ap=idx_t[:, cch:cch + 1], axis=0),
                )
                qrr += 1
                S_t = sel_pool.tile([P, W], f32, tag="sel")
                nc.vector.tensor_scalar(
                    out=S_t[:], in0=iota_t[:],
                    scalar1=dst_t[:, cch:cch + 1], scalar2=w_t[:, cch:cch + 1],
                    op0=mybir.AluOpType.is_equal, op1=mybir.AluOpType.mult,
                )
                g, cc2 = divmod(t, C)
                if cc2 == 0:
                    ps = psum_pool.tile([W, F], f32, tag="ps")
                nc.tensor.matmul(
                    out=ps[:], lhsT=S_t[:], rhs=rows_t[:, cch * F:(cch + 1) * F],
                    start=(cc2 == 0), stop=(cc2 == C - 1),
                )
                if cc2 == C - 1:
                    o_t = out_pool.tile([W, F], f32, tag="ot")
                    nc.vector.tensor_copy(out=o_t[:], in_=ps[:])
                    nc.sync.dma_start(
                        out=stag_ap[stag_off + g * W: stag_off + (g + 1) * W, :],
                        in_=o_t[:])

    with tile.TileContext(nc) as tc:
        with (
            tc.tile_pool(name="const", bufs=1) as const_pool,
            tc.tile_pool(name="meta", bufs=3) as meta_pool,
            tc.tile_pool(name="rows", bufs=3) as rows_pool,
            tc.tile_pool(name="sel", bufs=4) as sel_pool,
            tc.tile_pool(name="outp", bufs=4) as out_pool,
            tc.tile_pool(name="psum", bufs=8, space="PSUM") as psum_pool,
        ):
            iota_t = const_pool.tile([P, W], f32)
            nc.sync.dma_start(out=iota_t[:], in_=iota_d.ap()[:, :])
            pools = (meta_pool, rows_pool, sel_pool, out_pool, psum_pool, iota_t)

            for layer in (1, 2, 3):
                table_ap = (comb0_d.ap()[:, :] if layer == 1
                            else comb_ag[layer - 2].ap()[:, :])
                if layer == 3:
                    stag_ap = stag[2].ap()
                else:
                    stag_ap = stag_int[layer - 1].ap()
                emit_spmm(tc, pools, layer, "h", table_ap, stag_ap, 0)
                emit_spmm(tc, pools, layer, "g", table_ap, stag_ap, S_H)
                if layer < 3:
                    nc.gpsimd.collective_compute(
                        kind="AllGather", op=mybir.AluOpType.bypass,
                        replica_groups=[list(range(NCORES))],
                        ins=[stag_int[layer - 1].ap()[:, :]],
                        outs=[comb_ag[layer - 1].ap()[:, :]],
                    )
                    nc.sync.dma_start(out=stag[layer - 1].ap()[:, :],
                                      in_=stag_int[layer - 1].ap()[:, :])

    nc.compile()

    # ---------------- run (inline PJRT SPMD runner) ----------------
    import jax
    import jax.numpy as jnp
    from jax.sharding import Mesh, PartitionSpec, NamedSharding
    from jax.experimental.shard_map import shard_map
    from concourse import bass2jax
    from concourse.bass2jax import _bass_exec_p, partition_id_tensor

    bass2jax.install_neuronx_cc_hook()
    in_names, out_names, out_avals = [], [], []
    pname = nc.partition_id_tensor.name if nc.partition_id_tensor else None
    for alloc in nc.m.functions[0].allocations:
        if not isinstance(alloc, mybir.MemoryLocationSet):
            continue
        name = alloc.memorylocations[0].name
        if alloc.kind == "ExternalInput":
            if name != pname:
                in_names.append(name)
        elif alloc.kind == "ExternalOutput":
            out_names.append(name)
            out_avals.append(jax.core.ShapedArray(
                tuple(alloc.tensor_shape), mybir.dt.np(alloc.dtype)))
    n_params, n_outs = len(in_names), len(out_avals)
    all_in = list(in_names) + list(out_names) + ([pname] if pname else [])

    def _body(*args):
        operands = list(args)
        if pname is not None:
            operands.append(partition_id_tensor())
        return tuple(_bass_exec_p.bind(
            *operands, out_avals=tuple(out_avals), in_names=tuple(all_in),
            out_names=tuple(out_names), lowering_input_output_aliases=(),
            sim_require_finite=True, sim_require_nnan=True, nc=nc))

    devices = jax.devices()[:NCORES]
    mesh = Mesh(np.asarray(devices), ("core",))
    fn = jax.jit(
        shard_map(_body, mesh=mesh,
                  in_specs=(PartitionSpec("core"),) * (n_params + n_outs),
                  out_specs=(PartitionSpec("core"),) * n_outs),
        donate_argnums=tuple(range(n_params, n_params + n_outs)),
        keep_unused=True)

    sh = NamedSharding(mesh, PartitionSpec("core"))
    dev_in = [jax.device_put(
        np.concatenate([np.asarray(in_maps_np[c][n]) for c in range(NCORES)],
                       axis=0), sh) for n in in_names]

    def one_run():
        zo = [jax.device_put(jnp.zeros(
            (NCORES * av.shape[0], *av.shape[1:]), av.dtype), sh)
            for av in out_avals]
        out = fn(*dev_in, *zo)
        jax.block_until_ready(out)
        return out

    out_arrs = one_run()
    global LAST_EXEC_NS
    if os.environ.get("GNN_TIME", "1") != "0":
        times = []
        for _ in range(4):
            t0 = time.perf_counter()
            out_arrs = one_run()
            times.append(time.perf_counter() - t0)
        LAST_EXEC_NS = int(min(times) * 1e9)
        print(f"kernel exec wall times (ms): "
              + " ".join(f"{t*1e3:.2f}" for t in times))

    results = []
    for c in range(NCORES):
        d = {}
        for i, n in enumerate(out_names):
            av = out_avals[i]
            d[n] = np.asarray(out_arrs[i]).reshape(NCORES, *av.shape)[c]
        results.append(d)

    # ---------------- assemble outputs ----------------
    h_layers = [herbs_embedding]
    g_layers = [genes_embedding]
    for l in range(1, 4):
        key = f"stag{l}"
        h_full = np.empty((NH, F), np.float32)
        g_full = np.empty((NG, F), np.float32)
        for c in range(NCORES):
            st = results[c][key]
            h_full[c * HPC:(c + 1) * HPC] = st[cores[c]["h"]["loc"]]
            g_full[c * GPC:(c + 1) * GPC] = st[S_H + cores[c]["g"]["loc"]]
        h_layers.append(h_full)
        g_layers.append(g_full)

    gcn_herbs = np.concatenate(h_layers, axis=-1)
    gcn_genes = np.concatenate(g_layers, axis=-1)
    return gcn_herbs, gcn_genes
